# revision 9
# baseline (speedup 1.0000x reference)
"""LocalLinear (unfold + per-window Linear) Trainium2 Bass kernel.

Problem:
  x: [4096, 4096] f32
  W: [127, 128, 64] f32   (per-window Linear weight [out=128, in=64])
  b: [127, 128] f32
  out[bb, f*128+l] = sum_k x[bb, f*32+k] * W[f, l, k] + b[f, l]
  out: [4096, 16256] f32

Strategy:
  Data-parallel over batch across 8 NeuronCores (512 rows each).

  Host-side prep per core:
    - xta: x-shard transposed and packed into 64 K-tiles of 97 partitions.
      K-tile kt rows 0..95 hold x columns [64*kt, 64*kt+96), row 96 is
      constant 1.0 (bias row). Window stride is 32 and width 64, so folds
      2kt and 2kt+1 both lie fully inside K-tile kt (rows 0..63 / 32..95).
    - wr: block-banded weights. For K-tile kt, columns 0:128 are fold 2kt
      (W.T at rows 0..63, bias at row 96), columns 128:256 are fold 2kt+1
      (W.T at rows 32..95, bias at row 96); zeros elsewhere.

  Device (identical program on all cores, different data):
    - one matmul per (batch-tile, K-tile): out_psum = xta_tile.T @ wr_tile,
      [128 batch rows, 256 out cols], fp16 inputs, fp32 PSUM. Bias comes
      from ones-row x bias-row. No accumulation groups needed.
    - 4 K-tiles share one 2-bank PSUM tile; PSUM -> SBUF evacuation copies
      [128, 1024] at a time (cast to OUT_DT), alternating VectorE/ScalarE.
    - Large contiguous DMAs move the output shard to HBM.
"""

import threading

import numpy as np

# ---------------------------------------------------------------- constants
B = 4096          # batch
IN = 4096         # in_features
L = 128           # local_features
KW = 64           # kernel window
S = 32            # stride
F = 127           # fold_num
NCORES = 8
BS = B // NCORES  # 512 batch rows per core
NBT = BS // 128   # 4 batch tiles per core
NKT = 64          # K-tiles (stride 64, 2 folds each; last has 1)
KP = 97           # K-tile partitions: 96 data rows + 1 ones row
OUT_COLS = F * L  # 16256

IN_DT = np.float16   # matmul input dtype on device
OUT_DT = np.float16  # device output dtype (host casts back to f32)

# input DMA chunking (K-tiles per chunk; small first chunks for fast ramp)
IN_CHUNKS = [2, 2, 4, 8, 8, 8, 8, 8, 8, 8]
assert sum(IN_CHUNKS) == NKT
KT_PER_QUAD = 4              # K-tiles sharing one 2-bank PSUM tile
N_QUADS = NKT // KT_PER_QUAD           # 16 per batch tile
QUADS_PER_OUT_CHUNK = 4                # -> 4 output DMAs per batch tile

_cache_lock = threading.Lock()
_CACHE: dict = {}


def _build():
    """Build + compile the Bass program once per process."""
    import concourse.bacc as bacc
    import concourse.mybir as mybir
    import concourse.tile as tile

    in_dt = {2: mybir.dt.float16, 4: mybir.dt.float32}[np.dtype(IN_DT).itemsize]
    out_dt = {2: mybir.dt.float16, 4: mybir.dt.float32}[np.dtype(OUT_DT).itemsize]

    nc = bacc.Bacc(
        "TRN2",
        target_bir_lowering=False,
        debug=False,
        enable_asserts=False,
        num_devices=NCORES,
    )

    # Data rows ([96, N]) and the row-96 extras ([1, N]: ones / bias) ship as
    # separate DMAs: a 97-partition DMA degenerates to a single SDMA engine
    # (observed 26 GB/s), while 96- and 1-partition transfers fan out.
    xta_dram = nc.dram_tensor("xta", [KP - 1, NKT * BS], in_dt, kind="ExternalInput").ap()
    xb_dram = nc.dram_tensor("xb", [1, NKT * BS], in_dt, kind="ExternalInput").ap()
    wr_dram = nc.dram_tensor("wr", [KP - 1, NKT * 256], in_dt, kind="ExternalInput").ap()
    wb_dram = nc.dram_tensor("wb", [1, NKT * 256], in_dt, kind="ExternalInput").ap()
    out_dram = nc.dram_tensor("out", [BS, OUT_COLS], out_dt, kind="ExternalOutput").ap()

    with tile.TileContext(nc) as tc:
        with (
            tc.tile_pool(name="xin", bufs=1) as xin_pool,
            tc.tile_pool(name="win", bufs=1) as win_pool,
            tc.tile_pool(name="stage", bufs=3) as stage_pool,
            tc.tile_pool(name="psum", bufs=4, space="PSUM") as psum_pool,
        ):
            # ------------------------------------------------ input loads
            kt2chunk = {}
            xta_tiles = []
            wr_tiles = []
            kt0 = 0
            for c, nkt in enumerate(IN_CHUNKS):
                for k in range(nkt):
                    kt2chunk[kt0 + k] = (c, k)
                # inputs go through the ScalarE HWDGE ring so their issue
                # doesn't serialize behind output DMAs on the SP ring
                wr_t = win_pool.tile([KP, nkt * 256], in_dt,
                                     name=f"wr_c{c}", tag=f"wr_c{c}")
                nc.scalar.dma_start(
                    wr_t[:KP - 1, :], wr_dram[:, kt0 * 256:(kt0 + nkt) * 256])
                nc.scalar.dma_start(
                    wr_t[KP - 1:KP, :], wb_dram[:, kt0 * 256:(kt0 + nkt) * 256])
                xta_t = xin_pool.tile([KP, nkt * BS], in_dt,
                                      name=f"xta_c{c}", tag=f"xta_c{c}")
                nc.scalar.dma_start(
                    xta_t[:KP - 1, :], xta_dram[:, kt0 * BS:(kt0 + nkt) * BS])
                nc.scalar.dma_start(
                    xta_t[KP - 1:KP, :], xb_dram[:, kt0 * BS:(kt0 + nkt) * BS])
                wr_tiles.append(wr_t)
                xta_tiles.append(xta_t)
                kt0 += nkt

            # ------------------------------------------------ compute
            for t in range(NBT):
                stage_tiles = [
                    stage_pool.tile([128, QUADS_PER_OUT_CHUNK * 1024], out_dt,
                                    name=f"stage_t{t}_c{oc}", tag="stage")
                    for oc in range(N_QUADS // QUADS_PER_OUT_CHUNK)
                ]
                for q in range(N_QUADS):
                    psum_t = psum_pool.tile([128, 1024], mybir.dt.float32,
                                            name=f"ps_t{t}_q{q}", tag="ps")
                    for sub in range(KT_PER_QUAD):
                        kt = KT_PER_QUAD * q + sub
                        c, k = kt2chunk[kt]
                        ncols = 256 if kt < NKT - 1 else 128
                        lhsT = xta_tiles[c][:, k * BS + t * 128: k * BS + t * 128 + 128]
                        rhs = wr_tiles[c][:, k * 256: k * 256 + ncols]
                        nc.tensor.matmul(
                            psum_t[:, sub * 256: sub * 256 + ncols],
                            lhsT, rhs, start=True, stop=True)
                    # evacuate quad: out cols 1024q .. 1024q+qw of this row block
                    qw = 1024 if q < N_QUADS - 1 else 896
                    oc, qo = divmod(q, QUADS_PER_OUT_CHUNK)
                    dst = stage_tiles[oc][:, qo * 1024: qo * 1024 + qw]
                    if q % 2 == 0:
                        nc.vector.tensor_copy(dst, psum_t[:, :qw])
                    else:
                        nc.scalar.copy(dst, psum_t[:, :qw])
                    if qo == QUADS_PER_OUT_CHUNK - 1:
                        cw = QUADS_PER_OUT_CHUNK * 1024 - (1024 - qw)
                        nc.sync.dma_start(
                            out_dram[t * 128:(t + 1) * 128,
                                     oc * QUADS_PER_OUT_CHUNK * 1024:
                                     oc * QUADS_PER_OUT_CHUNK * 1024 + cw],
                            stage_tiles[oc][:, :cw])

    nc.compile()
    return nc


def _prepare_inputs(x, W, b):
    """Pack full inputs into 8 per-core input maps."""
    x = np.ascontiguousarray(np.asarray(x, dtype=np.float32))
    W = np.asarray(W, dtype=np.float32)
    b = np.asarray(b, dtype=np.float32)

    # wr is shared across cores: data rows [96, NKT, 256] + bias row [1, NKT, 256]
    wr = np.zeros((KP - 1, NKT, 256), dtype=np.float32)
    wb = np.zeros((1, NKT, 256), dtype=np.float32)
    for cb in range(2):  # column block: fold parity
        fs = np.arange(cb, F, 2)
        kts = fs // 2
        ro = 32 * cb
        # W[f]: [L, KW] -> want [KW(k rows), nf, L]
        wr[ro:ro + KW, kts, 128 * cb:128 * cb + L] = W[fs].transpose(2, 0, 1)
        wb[0, kts, 128 * cb:128 * cb + L] = b[fs]
    wr = np.ascontiguousarray(wr.reshape(KP - 1, NKT * 256).astype(IN_DT))
    wb = np.ascontiguousarray(wb.reshape(1, NKT * 256).astype(IN_DT))
    ones_row = np.ones((1, NKT * BS), dtype=IN_DT)

    xT = x.T  # [IN, B]
    # row indices per (partition p<96, kt): 64*kt + p, zero-padded past IN
    idx = (64 * np.arange(NKT)[None, :] + np.arange(KP - 1)[:, None])  # [96, NKT]
    pad_rows = int(idx.max()) + 1 - IN
    xT_pad = np.concatenate([xT, np.zeros((pad_rows, B), np.float32)], axis=0) \
        if pad_rows > 0 else xT

    in_maps = []
    for core in range(NCORES):
        cs = core * BS
        shard = xT_pad[:, cs:cs + BS]               # [IN+pad, BS]
        xta = np.ascontiguousarray(shard[idx].astype(IN_DT)
                                   .reshape(KP - 1, NKT * BS))  # [96, NKT*BS]
        in_maps.append({
            "xta": xta,
            "xb": ones_row,
            "wr": wr,
            "wb": wb,
        })
    return in_maps


def _get_nc():
    with _cache_lock:
        if "nc" not in _CACHE:
            _CACHE["nc"] = _build()
    return _CACHE["nc"]


def _run(in_maps, trace=False):
    from concourse.bass_utils import run_bass_kernel_spmd

    nc = _get_nc()
    res = run_bass_kernel_spmd(nc, in_maps, core_ids=list(range(NCORES)),
                               trace=trace)
    return res


def kernel(x, W, b):
    in_maps = _prepare_inputs(x, W, b)
    res = _run(in_maps, trace=False)
    out = np.concatenate([r["out"] for r in res.results], axis=0)
    return out.astype(np.float32)


# revision 10
# speedup vs baseline: 1.0551x; 1.0551x over previous
"""LocalLinear (unfold + per-window Linear) Trainium2 Bass kernel.

Problem:
  x: [4096, 4096] f32
  W: [127, 128, 64] f32   (per-window Linear weight [out=128, in=64])
  b: [127, 128] f32
  out[bb, f*128+l] = sum_k x[bb, f*32+k] * W[f, l, k] + b[f, l]
  out: [4096, 16256] f32

Strategy:
  Data-parallel over batch across 8 NeuronCores (512 rows each).

  Host-side prep per core:
    - xta: x-shard transposed and packed into 64 K-tiles of 97 partitions.
      K-tile kt rows 0..95 hold x columns [64*kt, 64*kt+96), row 96 is
      constant 1.0 (bias row). Window stride is 32 and width 64, so folds
      2kt and 2kt+1 both lie fully inside K-tile kt (rows 0..63 / 32..95).
    - wr: block-banded weights. For K-tile kt, columns 0:128 are fold 2kt
      (W.T at rows 0..63, bias at row 96), columns 128:256 are fold 2kt+1
      (W.T at rows 32..95, bias at row 96); zeros elsewhere.

  Device (identical program on all cores, different data):
    - one matmul per (batch-tile, K-tile): out_psum = xta_tile.T @ wr_tile,
      [128 batch rows, 256 out cols], fp16 inputs, fp32 PSUM. Bias comes
      from ones-row x bias-row. No accumulation groups needed.
    - 4 K-tiles share one 2-bank PSUM tile; PSUM -> SBUF evacuation copies
      [128, 1024] at a time (cast to OUT_DT), alternating VectorE/ScalarE.
    - Large contiguous DMAs move the output shard to HBM.
"""

import threading

import numpy as np

# ---------------------------------------------------------------- constants
B = 4096          # batch
IN = 4096         # in_features
L = 128           # local_features
KW = 64           # kernel window
S = 32            # stride
F = 127           # fold_num
NCORES = 8
BS = B // NCORES  # 512 batch rows per core
NBT = BS // 128   # 4 batch tiles per core
NKT = 64          # K-tiles (stride 64, 2 folds each; last has 1)
KP = 97           # K-tile partitions: 96 data rows + 1 ones row
OUT_COLS = F * L  # 16256

IN_DT = np.float16   # matmul input dtype on device
OUT_DT = np.float16  # device output dtype (host casts back to f32)

# input DMA chunking (K-tiles per chunk; small first chunks for fast ramp)
IN_CHUNKS = [2, 2, 4, 8, 8, 8, 8, 8, 8, 8]
assert sum(IN_CHUNKS) == NKT
KT_PER_QUAD = 4              # K-tiles sharing one 2-bank PSUM tile
N_QUADS = NKT // KT_PER_QUAD           # 16 per batch tile
QUADS_PER_OUT_CHUNK = 4                # -> 4 output DMAs per batch tile

_cache_lock = threading.Lock()
_CACHE: dict = {}


def _build():
    """Build + compile the Bass program once per process."""
    import concourse.bacc as bacc
    import concourse.mybir as mybir
    import concourse.tile as tile

    in_dt = {2: mybir.dt.float16, 4: mybir.dt.float32}[np.dtype(IN_DT).itemsize]
    out_dt = {2: mybir.dt.float16, 4: mybir.dt.float32}[np.dtype(OUT_DT).itemsize]

    nc = bacc.Bacc(
        "TRN2",
        target_bir_lowering=False,
        debug=False,
        enable_asserts=False,
        num_devices=NCORES,
    )

    # Data rows ([96, N]) and the row-96 extras ([1, N]: ones / bias) ship as
    # separate DMAs: a 97-partition DMA degenerates to a single SDMA engine
    # (observed 26 GB/s), while 96- and 1-partition transfers fan out.
    xta_dram = nc.dram_tensor("xta", [KP - 1, NKT * BS], in_dt, kind="ExternalInput").ap()
    xb_dram = nc.dram_tensor("xb", [1, NKT * BS], in_dt, kind="ExternalInput").ap()
    wr_dram = nc.dram_tensor("wr", [KP - 1, NKT * 256], in_dt, kind="ExternalInput").ap()
    wb_dram = nc.dram_tensor("wb", [1, NKT * 256], in_dt, kind="ExternalInput").ap()
    out_dram = nc.dram_tensor("out", [BS, OUT_COLS], out_dt, kind="ExternalOutput").ap()

    with tile.TileContext(nc) as tc:
        with (
            tc.tile_pool(name="xin", bufs=1) as xin_pool,
            tc.tile_pool(name="win", bufs=1) as win_pool,
            tc.tile_pool(name="stage", bufs=3) as stage_pool,
            tc.tile_pool(name="psum", bufs=4, space="PSUM") as psum_pool,
        ):
            # ------------------------------------------------ input loads
            kt2chunk = {}
            xta_tiles = []
            wr_tiles = []
            kt0 = 0
            for c, nkt in enumerate(IN_CHUNKS):
                for k in range(nkt):
                    kt2chunk[kt0 + k] = (c, k)
                wr_t = win_pool.tile([KP, nkt * 256], in_dt,
                                     name=f"wr_c{c}", tag=f"wr_c{c}")
                nc.sync.dma_start(
                    wr_t[:KP - 1, :], wr_dram[:, kt0 * 256:(kt0 + nkt) * 256])
                nc.sync.dma_start(
                    wr_t[KP - 1:KP, :], wb_dram[:, kt0 * 256:(kt0 + nkt) * 256])
                xta_t = xin_pool.tile([KP, nkt * BS], in_dt,
                                      name=f"xta_c{c}", tag=f"xta_c{c}")
                nc.sync.dma_start(
                    xta_t[:KP - 1, :], xta_dram[:, kt0 * BS:(kt0 + nkt) * BS])
                nc.sync.dma_start(
                    xta_t[KP - 1:KP, :], xb_dram[:, kt0 * BS:(kt0 + nkt) * BS])
                wr_tiles.append(wr_t)
                xta_tiles.append(xta_t)
                kt0 += nkt

            # ------------------------------------------------ compute
            for t in range(NBT):
                stage_tiles = [
                    stage_pool.tile([128, QUADS_PER_OUT_CHUNK * 1024], out_dt,
                                    name=f"stage_t{t}_c{oc}", tag="stage")
                    for oc in range(N_QUADS // QUADS_PER_OUT_CHUNK)
                ]
                for q in range(N_QUADS):
                    psum_t = psum_pool.tile([128, 1024], mybir.dt.float32,
                                            name=f"ps_t{t}_q{q}", tag="ps")
                    for sub in range(KT_PER_QUAD):
                        kt = KT_PER_QUAD * q + sub
                        c, k = kt2chunk[kt]
                        ncols = 256 if kt < NKT - 1 else 128
                        lhsT = xta_tiles[c][:, k * BS + t * 128: k * BS + t * 128 + 128]
                        rhs = wr_tiles[c][:, k * 256: k * 256 + ncols]
                        nc.tensor.matmul(
                            psum_t[:, sub * 256: sub * 256 + ncols],
                            lhsT, rhs, start=True, stop=True)
                    # evacuate quad: out cols 1024q .. 1024q+qw of this row block
                    qw = 1024 if q < N_QUADS - 1 else 896
                    oc, qo = divmod(q, QUADS_PER_OUT_CHUNK)
                    dst = stage_tiles[oc][:, qo * 1024: qo * 1024 + qw]
                    if q % 2 == 0:
                        nc.vector.tensor_copy(dst, psum_t[:, :qw])
                    else:
                        nc.scalar.copy(dst, psum_t[:, :qw])
                    if qo == QUADS_PER_OUT_CHUNK - 1:
                        cw = QUADS_PER_OUT_CHUNK * 1024 - (1024 - qw)
                        nc.sync.dma_start(
                            out_dram[t * 128:(t + 1) * 128,
                                     oc * QUADS_PER_OUT_CHUNK * 1024:
                                     oc * QUADS_PER_OUT_CHUNK * 1024 + cw],
                            stage_tiles[oc][:, :cw])

    nc.compile()
    return nc


def _prepare_inputs(x, W, b):
    """Pack full inputs into 8 per-core input maps."""
    x = np.ascontiguousarray(np.asarray(x, dtype=np.float32))
    W = np.asarray(W, dtype=np.float32)
    b = np.asarray(b, dtype=np.float32)

    # wr is shared across cores: data rows [96, NKT, 256] + bias row [1, NKT, 256]
    wr = np.zeros((KP - 1, NKT, 256), dtype=np.float32)
    wb = np.zeros((1, NKT, 256), dtype=np.float32)
    for cb in range(2):  # column block: fold parity
        fs = np.arange(cb, F, 2)
        kts = fs // 2
        ro = 32 * cb
        # W[f]: [L, KW] -> want [KW(k rows), nf, L]
        wr[ro:ro + KW, kts, 128 * cb:128 * cb + L] = W[fs].transpose(2, 0, 1)
        wb[0, kts, 128 * cb:128 * cb + L] = b[fs]
    wr = np.ascontiguousarray(wr.reshape(KP - 1, NKT * 256).astype(IN_DT))
    wb = np.ascontiguousarray(wb.reshape(1, NKT * 256).astype(IN_DT))
    ones_row = np.ones((1, NKT * BS), dtype=IN_DT)

    xT = x.T  # [IN, B]
    # row indices per (partition p<96, kt): 64*kt + p, zero-padded past IN
    idx = (64 * np.arange(NKT)[None, :] + np.arange(KP - 1)[:, None])  # [96, NKT]
    pad_rows = int(idx.max()) + 1 - IN
    xT_pad = np.concatenate([xT, np.zeros((pad_rows, B), np.float32)], axis=0) \
        if pad_rows > 0 else xT

    in_maps = []
    for core in range(NCORES):
        cs = core * BS
        shard = xT_pad[:, cs:cs + BS]               # [IN+pad, BS]
        xta = np.ascontiguousarray(shard[idx].astype(IN_DT)
                                   .reshape(KP - 1, NKT * BS))  # [96, NKT*BS]
        in_maps.append({
            "xta": xta,
            "xb": ones_row,
            "wr": wr,
            "wb": wb,
        })
    return in_maps


def _get_nc():
    with _cache_lock:
        if "nc" not in _CACHE:
            _CACHE["nc"] = _build()
    return _CACHE["nc"]


def _run(in_maps, trace=False):
    from concourse.bass_utils import run_bass_kernel_spmd

    nc = _get_nc()
    res = run_bass_kernel_spmd(nc, in_maps, core_ids=list(range(NCORES)),
                               trace=trace)
    return res


def kernel(x, W, b):
    in_maps = _prepare_inputs(x, W, b)
    res = _run(in_maps, trace=False)
    out = np.concatenate([r["out"] for r in res.results], axis=0)
    return out.astype(np.float32)
